# revision 12
# baseline (speedup 1.0000x reference)
"""ConvAttention Trainium2 kernel.

Problem (hardcoded shapes):
  x      [8, 64, 512, 128] f32
  W_qkv  [128, 256] f32
  qkv = x @ W_qkv ; q = qkv[..,:64], k = qkv[..,64:128], v = qkv[..,128:256]
  scores = q @ k^T over the W(=512) axis         [B,H,512,512]
  w = softmax(scores, -1)   (no max-subtraction needed: |scores| < ~50)
  y = w @ v + x
  returns (y, w)

Sharding: data-parallel over B across the 8 cores (core c <- batch b=c, all
64 h). W_qkv replicated.

Per-(b,h) device algorithm (all matmuls contract over partitions):
  A = Wq @ Wk^T  [128,128] (once, fp32)  =>  scores = (x A) x^T
  xT  = PE-transpose(x)                       [C=128, 512]   (fp32r rounded)
  uT  = A^T... matmul(lhsT=A_r, rhs=xT)       [128, 512]     (= (xA)^T)
  v   = matmul(lhsT=xT[:,chunk], rhs=Wv_r)    4x [128,128] -> v' bf16 + ones col
  scores  (2 halves) = matmul(lhsT=uT[:,qc], rhs=xT)  [128, 2, 512] psum
  scoresT (2 halves) = matmul(lhsT=xT[:,jc], rhs=uT)
  exp1: ACT Exp psum->sbuf fp32  (w unnormalized, [q,j] layout)
  exp2: ACT Exp psum->sbuf bf16  (w unnorm transposed, [j,q] layout)
  AV:   y_psum[:,d,0:129] = sum_jc w_unT[jc][:,qd] @ [v'|ones]  (bf16, fp32 acc)
        -> col 128 is Z (softmax denominator) for free
  zr = 1/Z;  w = w_un * zr (DVE);  y = y_un * zr + x (DVE scalar_tensor_tensor)
"""

import numpy as np

import concourse.bacc as bacc
import concourse.tile as tile
from concourse import mybir
from concourse import bass_utils
from concourse.masks import make_identity

B, H, W, C = 8, 64, 512, 128
NCORES = 8
NBH = H  # 64 (b,h) pairs per core
NQC = 4  # 512 = 4 chunks of 128
FP32 = mybir.dt.float32
FP32R = mybir.dt.float32r
BF16 = mybir.dt.bfloat16

_CACHE = {}


def build(reps: int = 1):
    nc = bacc.Bacc("TRN2", target_bir_lowering=False, debug=False)
    x_d = nc.dram_tensor("x", [NBH, W, C], FP32, kind="ExternalInput")
    wq_d = nc.dram_tensor("W_qkv", [C, 2 * C], FP32, kind="ExternalInput")
    w_d = nc.dram_tensor("w_out", [NBH, W, W], FP32, kind="ExternalOutput")
    y_d = nc.dram_tensor("y_out", [NBH, W, C], FP32, kind="ExternalOutput")

    mult = mybir.AluOpType.mult
    add = mybir.AluOpType.add
    EXP = mybir.ActivationFunctionType.Exp

    with tile.TileContext(nc) as tc:
        with (
            tc.tile_pool(name="singles", bufs=1) as singles,
            tc.tile_pool(name="sb", bufs=4) as sb,
            tc.tile_pool(name="wpool", bufs=4) as wpool,
            tc.tile_pool(name="pp_small", bufs=2, space="PSUM") as pp_small,
            tc.tile_pool(name="pp_sc", bufs=2, space="PSUM") as pp_sc,
            tc.tile_pool(name="pp_y", bufs=1, space="PSUM") as pp_y,
        ):
            # ---- one-time setup ----
            w_sb = singles.tile([C, 2 * C], FP32)
            nc.sync.dma_start(out=w_sb, in_=wq_d.ap())
            ident = singles.tile([C, C], FP32)
            make_identity(nc, ident)

            wqT_ps = pp_small.tile([C, W], FP32, tag="small")
            nc.tensor.transpose(wqT_ps[0:64, 0:C], w_sb[:, 0:64], ident)
            nc.tensor.transpose(wqT_ps[0:64, C : 2 * C], w_sb[:, 64:C], ident)
            wqkT = singles.tile([64, 2, C], FP32)
            nc.vector.tensor_copy(wqkT, wqT_ps[0:64, 0 : 2 * C])

            a_ps = pp_small.tile([C, W], FP32, tag="small")
            nc.tensor.matmul(
                a_ps[:, 0:C], wqkT[:, 0, :], wqkT[:, 1, :], start=True, stop=True
            )
            a_r = singles.tile([C, C], FP32R)
            nc.vector.tensor_copy(a_r, a_ps[:, 0:C])
            wv_bf = singles.tile([C, C], BF16)
            nc.vector.tensor_copy(wv_bf, w_sb[:, C : 2 * C])

            # ---- per-(b,h) pipeline ----
            import contextlib

            rep_ctx = tc.For_i(0, reps, 1) if reps > 1 else contextlib.nullcontext()
            with rep_ctx:
              for i in range(NBH):
                x_nat = sb.tile([C, NQC, C], FP32)
                nc.scalar.dma_start(
                    out=x_nat, in_=x_d.ap()[i].rearrange("(c p) ch -> p c ch", p=C)
                )

                xT_ps = pp_small.tile([C, W], FP32, tag="small")
                for c in range(NQC):
                    nc.tensor.transpose(
                        xT_ps[:, c * C : (c + 1) * C], x_nat[:, c, :], ident
                    )
                xT = sb.tile([C, W], FP32R)
                nc.vector.tensor_copy(xT, xT_ps)
                xT_bf = sb.tile([C, W], BF16)
                nc.vector.tensor_copy(xT_bf, xT_ps)

                uT_ps = pp_small.tile([C, W], FP32, tag="small")
                nc.tensor.matmul(uT_ps, a_r, xT, start=True, stop=True)
                uT = sb.tile([C, W], FP32R)
                nc.vector.tensor_copy(uT, uT_ps)

                v_ps = pp_small.tile([C, W], FP32, tag="small")
                for c in range(NQC):
                    nc.tensor.matmul(
                        v_ps[:, c * C : (c + 1) * C],
                        xT_bf[:, c * C : (c + 1) * C],
                        wv_bf,
                        start=True,
                        stop=True,
                    )
                v1 = sb.tile([C, NQC, 132], BF16)
                nc.vector.tensor_copy(
                    v1[:, :, 0:C], v_ps.rearrange("p (c ch) -> p c ch", c=NQC)
                )
                nc.gpsimd.memset(v1[:, :, C : C + 1], 1.0)

                w_un = wpool.tile([C, NQC, W], FP32)
                w_unT = sb.tile([C, NQC, W], BF16)
                for h in range(2):
                    s_ps = pp_sc.tile([C, 2, W], FP32, tag="sc")
                    for k in range(2):
                        qc = 2 * h + k
                        nc.tensor.matmul(
                            s_ps[:, k, :],
                            uT[:, qc * C : (qc + 1) * C],
                            xT,
                            start=True,
                            stop=True,
                        )
                    nc.scalar.activation(w_un[:, 2 * h : 2 * h + 2, :], s_ps, EXP)

                    sT_ps = pp_sc.tile([C, 2, W], FP32, tag="sc")
                    for k in range(2):
                        jc = 2 * h + k
                        nc.tensor.matmul(
                            sT_ps[:, k, :],
                            xT[:, jc * C : (jc + 1) * C],
                            uT,
                            start=True,
                            stop=True,
                        )
                    nc.scalar.activation(w_unT[:, 2 * h : 2 * h + 2, :], sT_ps, EXP)

                y_ps = pp_y.tile([C, NQC, 256], FP32)
                for d in range(NQC):
                    for jc in range(NQC):
                        nc.tensor.matmul(
                            y_ps[:, d, 0:129],
                            w_unT[:, jc, d * C : (d + 1) * C],
                            v1[:, jc, 0:129],
                            start=(jc == 0),
                            stop=(jc == NQC - 1),
                        )

                zr = sb.tile([C, NQC], FP32)
                nc.vector.reciprocal(zr, y_ps[:, :, 128])

                w_norm = wpool.tile([C, NQC, W], FP32)
                for c in range(NQC):
                    nc.vector.tensor_scalar_mul(
                        out=w_norm[:, c, :], in0=w_un[:, c, :], scalar1=zr[:, c : c + 1]
                    )

                y_sb = sb.tile([C, NQC, C], FP32)
                for d in range(NQC):
                    nc.vector.scalar_tensor_tensor(
                        out=y_sb[:, d, :],
                        in0=y_ps[:, d, 0:C],
                        scalar=zr[:, d : d + 1],
                        in1=x_nat[:, d, :],
                        op0=mult,
                        op1=add,
                    )

                nc.sync.dma_start(
                    out=w_d.ap()[i].rearrange("(c p) j -> p c j", p=C), in_=w_norm
                )
                nc.scalar.dma_start(
                    out=y_d.ap()[i].rearrange("(c p) ch -> p c ch", p=C), in_=y_sb
                )

    nc.compile()
    return nc


def _get_nc(reps: int = 1):
    key = ("nc", reps)
    if key not in _CACHE:
        _CACHE[key] = build(reps)
    return _CACHE[key]


def kernel(x: np.ndarray, W_qkv: np.ndarray, trace: bool = False):
    nc = _get_nc()
    x = np.ascontiguousarray(x, dtype=np.float32)
    W_qkv = np.ascontiguousarray(W_qkv, dtype=np.float32)
    in_maps = [{"x": x[c], "W_qkv": W_qkv} for c in range(NCORES)]
    res = bass_utils.run_bass_kernel_spmd(
        nc, in_maps, core_ids=list(range(NCORES)), trace=trace
    )
    y = np.stack([res.results[c]["y_out"] for c in range(NCORES)])
    w = np.stack([res.results[c]["w_out"] for c in range(NCORES)])
    if trace:
        _CACHE["last_results"] = res
    return (y, w)


# revision 20
# speedup vs baseline: 1.0342x; 1.0342x over previous
"""ConvAttention Trainium2 kernel.

Problem (hardcoded shapes):
  x      [8, 64, 512, 128] f32
  W_qkv  [128, 256] f32
  qkv = x @ W_qkv ; q = qkv[..,:64], k = qkv[..,64:128], v = qkv[..,128:256]
  scores = q @ k^T over the W(=512) axis         [B,H,512,512]
  w = softmax(scores, -1)   (no max-subtraction needed: |scores| < ~50)
  y = w @ v + x
  returns (y, w)

Sharding: data-parallel over B across the 8 cores (core c <- batch b=c, all
64 h). W_qkv replicated.

Per-(b,h) device algorithm (all matmuls contract over partitions):
  A = Wq @ Wk^T  [128,128] (once, fp32)  =>  scores = (x A) x^T
  xT  = PE-transpose(x)                       [C=128, 512]   (fp32r rounded)
  uT  = A^T... matmul(lhsT=A_r, rhs=xT)       [128, 512]     (= (xA)^T)
  v   = matmul(lhsT=xT[:,chunk], rhs=Wv_r)    4x [128,128] -> v' bf16 + ones col
  scores  (2 halves) = matmul(lhsT=uT[:,qc], rhs=xT)  [128, 2, 512] psum
  scoresT (2 halves) = matmul(lhsT=xT[:,jc], rhs=uT)
  exp1: ACT Exp psum->sbuf fp32  (w unnormalized, [q,j] layout)
  exp2: ACT Exp psum->sbuf bf16  (w unnorm transposed, [j,q] layout)
  AV:   y_psum[:,d,0:129] = sum_jc w_unT[jc][:,qd] @ [v'|ones]  (bf16, fp32 acc)
        -> col 128 is Z (softmax denominator) for free
  zr = 1/Z;  w = w_un * zr (DVE);  y = y_un * zr + x (DVE scalar_tensor_tensor)
"""

import numpy as np

import concourse.bacc as bacc
import concourse.tile as tile
from concourse import mybir
from concourse import bass_utils
from concourse.masks import make_identity

B, H, W, C = 8, 64, 512, 128
NCORES = 8
NBH = H  # 64 (b,h) pairs per core
NQC = 4  # 512 = 4 chunks of 128
FP32 = mybir.dt.float32
FP32R = mybir.dt.float32r
BF16 = mybir.dt.bfloat16

_CACHE = {}


def build(reps: int = 1, ablate: frozenset = frozenset()):
    nc = bacc.Bacc("TRN2", target_bir_lowering=False, debug=False)
    x_d = nc.dram_tensor("x", [NBH, W, C], FP32, kind="ExternalInput")
    wq_d = nc.dram_tensor("W_qkv", [C, 2 * C], FP32, kind="ExternalInput")
    w_d = nc.dram_tensor("w_out", [NBH, W, W], BF16, kind="ExternalOutput")
    y_d = nc.dram_tensor("y_out", [NBH, W, C], FP32, kind="ExternalOutput")

    mult = mybir.AluOpType.mult
    add = mybir.AluOpType.add
    EXP = mybir.ActivationFunctionType.Exp

    with tile.TileContext(nc) as tc:
        with (
            tc.tile_pool(name="singles", bufs=1) as singles,
            tc.tile_pool(name="sb", bufs=4) as sb,
            tc.tile_pool(name="wpool", bufs=4) as wpool,
            tc.tile_pool(name="pp_small", bufs=2, space="PSUM") as pp_small,
            tc.tile_pool(name="pp_sc", bufs=2, space="PSUM") as pp_sc,
            tc.tile_pool(name="pp_y", bufs=1, space="PSUM") as pp_y,
        ):
            # ---- one-time setup ----
            w_sb = singles.tile([C, 2 * C], FP32)
            nc.sync.dma_start(out=w_sb, in_=wq_d.ap())
            ident = singles.tile([C, C], FP32)
            make_identity(nc, ident)

            wqT_ps = pp_small.tile([C, W], FP32, tag="small")
            nc.tensor.transpose(wqT_ps[0:64, 0:C], w_sb[:, 0:64], ident)
            nc.tensor.transpose(wqT_ps[0:64, C : 2 * C], w_sb[:, 64:C], ident)
            wqkT = singles.tile([64, 2, C], FP32)
            nc.vector.tensor_copy(wqkT, wqT_ps[0:64, 0 : 2 * C])

            a_ps = pp_small.tile([C, W], FP32, tag="small")
            nc.tensor.matmul(
                a_ps[:, 0:C], wqkT[:, 0, :], wqkT[:, 1, :], start=True, stop=True
            )
            a_r = singles.tile([C, C], FP32R)
            nc.vector.tensor_copy(a_r, a_ps[:, 0:C])
            wv_r = singles.tile([C, C], FP32R)
            nc.vector.tensor_copy(wv_r, w_sb[:, C : 2 * C])

            # ---- per-(b,h) pipeline ----
            import contextlib

            rep_ctx = tc.For_i(0, reps, 1) if reps > 1 else contextlib.nullcontext()
            with rep_ctx:
              for i in range(NBH):
                x_nat = sb.tile([C, NQC, C], FP32)
                if "xdma" not in ablate:
                    nc.scalar.dma_start(
                        out=x_nat, in_=x_d.ap()[i].rearrange("(c p) ch -> p c ch", p=C)
                    )
                else:
                    nc.gpsimd.memset(x_nat, 0.01)

                xT_ps = pp_small.tile([C, W], FP32, tag="small")
                for c in range(NQC):
                    nc.tensor.transpose(
                        xT_ps[:, c * C : (c + 1) * C], x_nat[:, c, :], ident
                    )
                xT = sb.tile([C, W], FP32R)
                nc.vector.tensor_copy(xT, xT_ps)

                uT_ps = pp_small.tile([C, W], FP32, tag="small")
                nc.tensor.matmul(uT_ps, a_r, xT, start=True, stop=True)
                uT = sb.tile([C, W], FP32R)
                nc.vector.tensor_copy(uT, uT_ps)

                v_ps = pp_small.tile([C, W], FP32, tag="small")
                for c in range(NQC):
                    nc.tensor.matmul(
                        v_ps[:, c * C : (c + 1) * C],
                        xT[:, c * C : (c + 1) * C],
                        wv_r,
                        start=True,
                        stop=True,
                    )
                v1 = sb.tile([C, NQC, 132], BF16)
                nc.vector.tensor_copy(
                    v1[:, :, 0:C], v_ps.rearrange("p (c ch) -> p c ch", c=NQC)
                )
                nc.gpsimd.memset(v1[:, :, C : C + 1], 1.0)

                w_un = wpool.tile([C, NQC, W], BF16)
                w_unT = sb.tile([C, NQC, W], BF16)
                for h in range(2):
                    s_ps = pp_sc.tile([C, 2, W], FP32, tag="sc")
                    for k in range(2):
                        qc = 2 * h + k
                        for _rep in range(2 if "2xscores" in ablate else 1):
                            nc.tensor.matmul(
                                s_ps[:, k, :],
                                uT[:, qc * C : (qc + 1) * C],
                                xT,
                                start=True,
                                stop=True,
                            )
                    for _rep in range(2 if "2xexp" in ablate else 1):
                        nc.scalar.activation(w_un[:, 2 * h : 2 * h + 2, :], s_ps, EXP)

                    sT_ps = pp_sc.tile([C, 2, W], FP32, tag="sc")
                    for k in range(2):
                        jc = 2 * h + k
                        for _rep in range(2 if "2xscores" in ablate else 1):
                            nc.tensor.matmul(
                                sT_ps[:, k, :],
                                xT[:, jc * C : (jc + 1) * C],
                                uT,
                                start=True,
                                stop=True,
                            )
                    for _rep in range(2 if "2xexp" in ablate else 1):
                        nc.scalar.activation(w_unT[:, 2 * h : 2 * h + 2, :], sT_ps, EXP)

                y_ps = pp_y.tile([C, NQC, 256], FP32)
                for d in range(NQC):
                    for jc in range(NQC):
                        nc.tensor.matmul(
                            y_ps[:, d, 0:129],
                            w_unT[:, jc, d * C : (d + 1) * C],
                            v1[:, jc, 0:129],
                            start=(jc == 0),
                            stop=(jc == NQC - 1),
                        )

                zr = sb.tile([C, NQC], FP32)
                nc.vector.reciprocal(zr, y_ps[:, :, 128])

                w_norm = wpool.tile([C, NQC, W], BF16)
                for _rep in range(2 if "2xwnorm" in ablate else 1):
                    for c in range(NQC):
                        nc.vector.tensor_scalar_mul(
                            out=w_norm[:, c, :], in0=w_un[:, c, :], scalar1=zr[:, c : c + 1]
                        )

                y_sb = sb.tile([C, NQC, C], FP32)
                for d in range(NQC):
                    nc.vector.scalar_tensor_tensor(
                        out=y_sb[:, d, :],
                        in0=y_ps[:, d, 0:C],
                        scalar=zr[:, d : d + 1],
                        in1=x_nat[:, d, :],
                        op0=mult,
                        op1=add,
                    )

                for _rep in range(2 if "2xwdma" in ablate else 1):
                    nc.sync.dma_start(
                        out=w_d.ap()[i].rearrange("(c p) j -> p c j", p=C), in_=w_norm
                    )
                nc.scalar.dma_start(
                    out=y_d.ap()[i].rearrange("(c p) ch -> p c ch", p=C), in_=y_sb
                )

    nc.compile()
    return nc


def _get_nc(reps: int = 1, ablate: frozenset = frozenset()):
    key = ("nc", reps, ablate)
    if key not in _CACHE:
        _CACHE[key] = build(reps, ablate)
    return _CACHE[key]


def kernel(x: np.ndarray, W_qkv: np.ndarray, trace: bool = False):
    nc = _get_nc()
    x = np.ascontiguousarray(x, dtype=np.float32)
    W_qkv = np.ascontiguousarray(W_qkv, dtype=np.float32)
    in_maps = [{"x": x[c], "W_qkv": W_qkv} for c in range(NCORES)]
    res = bass_utils.run_bass_kernel_spmd(
        nc, in_maps, core_ids=list(range(NCORES)), trace=trace
    )
    y = np.stack([res.results[c]["y_out"] for c in range(NCORES)])
    w = np.stack(
        [res.results[c]["w_out"] for c in range(NCORES)]
    ).astype(np.float32)
    if trace:
        _CACHE["last_results"] = res
    return (y, w)


# revision 21
# speedup vs baseline: 1.2143x; 1.1742x over previous
"""ConvAttention Trainium2 kernel.

Problem (hardcoded shapes):
  x      [8, 64, 512, 128] f32
  W_qkv  [128, 256] f32
  qkv = x @ W_qkv ; q = qkv[..,:64], k = qkv[..,64:128], v = qkv[..,128:256]
  scores = q @ k^T over the W(=512) axis         [B,H,512,512]
  w = softmax(scores, -1)   (no max-subtraction needed: |scores| < ~50)
  y = w @ v + x
  returns (y, w)

Sharding: data-parallel over B across the 8 cores (core c <- batch b=c, all
64 h). W_qkv replicated.

Per-(b,h) device algorithm (all matmuls contract over partitions):
  A = Wq @ Wk^T  [128,128] (once, fp32)  =>  scores = (x A) x^T
  xT  = PE-transpose(x)                       [C=128, 512]   (fp32r rounded)
  uT  = A^T... matmul(lhsT=A_r, rhs=xT)       [128, 512]     (= (xA)^T)
  v   = matmul(lhsT=xT[:,chunk], rhs=Wv_r)    4x [128,128] -> v' bf16 + ones col
  scores  (2 halves) = matmul(lhsT=uT[:,qc], rhs=xT)  [128, 2, 512] psum
  scoresT (2 halves) = matmul(lhsT=xT[:,jc], rhs=uT)
  exp1: ACT Exp psum->sbuf fp32  (w unnormalized, [q,j] layout)
  exp2: ACT Exp psum->sbuf bf16  (w unnorm transposed, [j,q] layout)
  AV:   y_psum[:,d,0:129] = sum_jc w_unT[jc][:,qd] @ [v'|ones]  (bf16, fp32 acc)
        -> col 128 is Z (softmax denominator) for free
  zr = 1/Z;  w = w_un * zr (DVE);  y = y_un * zr + x (DVE scalar_tensor_tensor)
"""

import numpy as np

import concourse.bacc as bacc
import concourse.tile as tile
from concourse import mybir
from concourse import bass_utils
from concourse.masks import make_identity

B, H, W, C = 8, 64, 512, 128
NCORES = 8
NBH = H  # 64 (b,h) pairs per core
NQC = 4  # 512 = 4 chunks of 128
FP32 = mybir.dt.float32
FP32R = mybir.dt.float32r
BF16 = mybir.dt.bfloat16

_CACHE = {}


def build(reps: int = 1, ablate: frozenset = frozenset()):
    nc = bacc.Bacc("TRN2", target_bir_lowering=False, debug=False)
    x_d = nc.dram_tensor("x", [NBH, W, C], FP32, kind="ExternalInput")
    wq_d = nc.dram_tensor("W_qkv", [C, 2 * C], FP32, kind="ExternalInput")
    w_d = nc.dram_tensor("w_out", [NBH, W, W], BF16, kind="ExternalOutput")
    y_d = nc.dram_tensor("y_out", [NBH, W, C], FP32, kind="ExternalOutput")

    mult = mybir.AluOpType.mult
    add = mybir.AluOpType.add
    EXP = mybir.ActivationFunctionType.Exp

    with tile.TileContext(nc) as tc:
        with (
            tc.tile_pool(name="singles", bufs=1) as singles,
            tc.tile_pool(name="sb", bufs=6) as sb,
            tc.tile_pool(name="wpool", bufs=6) as wpool,
            tc.tile_pool(name="pp_small", bufs=2, space="PSUM") as pp_small,
            tc.tile_pool(name="pp_sc", bufs=2, space="PSUM") as pp_sc,
            tc.tile_pool(name="pp_y", bufs=1, space="PSUM") as pp_y,
        ):
            # ---- one-time setup ----
            w_sb = singles.tile([C, 2 * C], FP32)
            nc.sync.dma_start(out=w_sb, in_=wq_d.ap())
            ident = singles.tile([C, C], FP32)
            make_identity(nc, ident)

            wqT_ps = pp_small.tile([C, W], FP32, tag="small")
            nc.tensor.transpose(wqT_ps[0:64, 0:C], w_sb[:, 0:64], ident)
            nc.tensor.transpose(wqT_ps[0:64, C : 2 * C], w_sb[:, 64:C], ident)
            wqkT = singles.tile([64, 2, C], FP32)
            nc.vector.tensor_copy(wqkT, wqT_ps[0:64, 0 : 2 * C])

            a_ps = pp_small.tile([C, W], FP32, tag="small")
            nc.tensor.matmul(
                a_ps[:, 0:C], wqkT[:, 0, :], wqkT[:, 1, :], start=True, stop=True
            )
            a_r = singles.tile([C, C], FP32R)
            nc.vector.tensor_copy(a_r, a_ps[:, 0:C])
            wv_r = singles.tile([C, C], FP32R)
            nc.vector.tensor_copy(wv_r, w_sb[:, C : 2 * C])

            # ---- per-(b,h) pipeline ----
            import contextlib

            rep_ctx = tc.For_i(0, reps, 1) if reps > 1 else contextlib.nullcontext()
            with rep_ctx:
              for i in range(NBH):
                x_nat = sb.tile([C, NQC, C], FP32)
                if "xdma" not in ablate:
                    nc.scalar.dma_start(
                        out=x_nat, in_=x_d.ap()[i].rearrange("(c p) ch -> p c ch", p=C)
                    )
                else:
                    nc.gpsimd.memset(x_nat, 0.01)

                xT_ps = pp_small.tile([C, W], FP32, tag="small")
                for c in range(NQC):
                    nc.tensor.transpose(
                        xT_ps[:, c * C : (c + 1) * C], x_nat[:, c, :], ident
                    )
                xT = sb.tile([C, W], FP32R)
                nc.vector.tensor_copy(xT, xT_ps)

                uT_ps = pp_small.tile([C, W], FP32, tag="small")
                nc.tensor.matmul(uT_ps, a_r, xT, start=True, stop=True)
                uT = sb.tile([C, W], FP32R)
                nc.vector.tensor_copy(uT, uT_ps)

                v_ps = pp_small.tile([C, W], FP32, tag="small")
                for c in range(NQC):
                    nc.tensor.matmul(
                        v_ps[:, c * C : (c + 1) * C],
                        xT[:, c * C : (c + 1) * C],
                        wv_r,
                        start=True,
                        stop=True,
                    )
                v1 = sb.tile([C, NQC, 132], BF16)
                nc.vector.tensor_copy(
                    v1[:, :, 0:C], v_ps.rearrange("p (c ch) -> p c ch", c=NQC)
                )
                nc.gpsimd.memset(v1[:, :, C : C + 1], 1.0)

                w_un = wpool.tile([C, NQC, W], BF16)
                w_unT = sb.tile([C, NQC, W], BF16)
                for h in range(2):
                    s_ps = pp_sc.tile([C, 2, W], FP32, tag="sc")
                    for k in range(2):
                        qc = 2 * h + k
                        for _rep in range(2 if "2xscores" in ablate else 1):
                            nc.tensor.matmul(
                                s_ps[:, k, :],
                                uT[:, qc * C : (qc + 1) * C],
                                xT,
                                start=True,
                                stop=True,
                            )
                    sT_ps = pp_sc.tile([C, 2, W], FP32, tag="sc")
                    for k in range(2):
                        jc = 2 * h + k
                        for _rep in range(2 if "2xscores" in ablate else 1):
                            nc.tensor.matmul(
                                sT_ps[:, k, :],
                                xT[:, jc * C : (jc + 1) * C],
                                uT,
                                start=True,
                                stop=True,
                            )
                    for _rep in range(2 if "2xexp" in ablate else 1):
                        nc.scalar.activation(w_un[:, 2 * h : 2 * h + 2, :], s_ps, EXP)
                    for _rep in range(2 if "2xexp" in ablate else 1):
                        nc.scalar.activation(w_unT[:, 2 * h : 2 * h + 2, :], sT_ps, EXP)

                y_ps = pp_y.tile([C, NQC, 256], FP32)
                for d in range(NQC):
                    for jc in range(NQC):
                        nc.tensor.matmul(
                            y_ps[:, d, 0:129],
                            w_unT[:, jc, d * C : (d + 1) * C],
                            v1[:, jc, 0:129],
                            start=(jc == 0),
                            stop=(jc == NQC - 1),
                        )

                zr = sb.tile([C, NQC], FP32)
                nc.vector.reciprocal(zr, y_ps[:, :, 128])

                w_norm = wpool.tile([C, NQC, W], BF16)
                for _rep in range(2 if "2xwnorm" in ablate else 1):
                    for c in range(NQC):
                        nc.vector.tensor_scalar_mul(
                            out=w_norm[:, c, :], in0=w_un[:, c, :], scalar1=zr[:, c : c + 1]
                        )

                y_sb = sb.tile([C, NQC, C], FP32)
                for d in range(NQC):
                    nc.vector.scalar_tensor_tensor(
                        out=y_sb[:, d, :],
                        in0=y_ps[:, d, 0:C],
                        scalar=zr[:, d : d + 1],
                        in1=x_nat[:, d, :],
                        op0=mult,
                        op1=add,
                    )

                for _rep in range(2 if "2xwdma" in ablate else 1):
                    nc.sync.dma_start(
                        out=w_d.ap()[i].rearrange("(c p) j -> p c j", p=C), in_=w_norm
                    )
                nc.scalar.dma_start(
                    out=y_d.ap()[i].rearrange("(c p) ch -> p c ch", p=C), in_=y_sb
                )

    nc.compile()
    return nc


def _get_nc(reps: int = 1, ablate: frozenset = frozenset()):
    key = ("nc", reps, ablate)
    if key not in _CACHE:
        _CACHE[key] = build(reps, ablate)
    return _CACHE[key]


def kernel(x: np.ndarray, W_qkv: np.ndarray, trace: bool = False):
    nc = _get_nc()
    x = np.ascontiguousarray(x, dtype=np.float32)
    W_qkv = np.ascontiguousarray(W_qkv, dtype=np.float32)
    in_maps = [{"x": x[c], "W_qkv": W_qkv} for c in range(NCORES)]
    res = bass_utils.run_bass_kernel_spmd(
        nc, in_maps, core_ids=list(range(NCORES)), trace=trace
    )
    y = np.stack([res.results[c]["y_out"] for c in range(NCORES)])
    w = np.stack(
        [res.results[c]["w_out"] for c in range(NCORES)]
    ).astype(np.float32)
    if trace:
        _CACHE["last_results"] = res
    return (y, w)
